# revision 1
# baseline (speedup 1.0000x reference)
"""AssignAttention (topk_masking) Trainium2 kernel — 8 NeuronCores.

Sharding: data-parallel over B (2 groups of 4 cores), tensor-parallel over
heads H (2 heads per core). Per core: QKV projections for its 2 heads (f32r
matmuls), rawT = k q^T per head, per-column top-4 via the DVE MAX8
instruction, binary Y^T mask, count matmuls, sparse-softmax reformulation
(exp(A) = 1 + (e_n-1) Y), Yv matmul, output projection with fused
normalization, ReduceScatter(add) over each 4-core group.

Math: with Y[n,s] = [n in top4 of column s], c_n = sum_s Y, cm_n = sum_s
mask_s Y, e_n = exp(1/(c_n+1)), M = sum_s mask_s, Z_n = M + (e_n-1) cm_n:
out_head[n,:] = (Vsum + (e_n-1) * (Y.mask @ v)[n,:]) / Z_n
which equals the reference's hard-topk + sum-normalize + masked softmax.
"""
import sys, os

os.environ["JAX_ENABLE_COMPILATION_CACHE"] = "false"
sys.path.insert(0, "/opt/trn_rl_repo")
import numpy as np
import ml_dtypes

B, N, C, H, K = 2, 2048, 1024, 8, 4
HD = C // H
SCALE = HD ** -0.5
NCORES = 8
ST = 16          # s-tiles per head
F32 = None       # set after import
BF16 = None

_cache = {}


def _build():
    from concourse import bacc, tile, mybir

    f32, f32r, bf16 = mybir.dt.float32, mybir.dt.float32r, mybir.dt.bfloat16
    AF = mybir.ActivationFunctionType
    OP = mybir.AluOpType

    nc = bacc.Bacc(None, target_bir_lowering=False)
    d_qt = nc.declare_dram_parameter("qt", [C, N], f32, isOutput=False)
    d_wq = nc.declare_dram_parameter("wq", [C, 2 * HD], f32, isOutput=False)
    d_wk = nc.declare_dram_parameter("wk", [C, 2 * HD], f32, isOutput=False)
    d_wv = nc.declare_dram_parameter("wv", [C, 2 * HD], f32, isOutput=False)
    d_wp = nc.declare_dram_parameter("wp", [2 * HD, C], f32, isOutput=False)
    d_maskT = nc.declare_dram_parameter("maskT", [128, ST], f32, isOutput=False)
    d_om = nc.declare_dram_parameter("om", [128, 2 * ST], bf16, isOutput=False)
    d_ones = nc.declare_dram_parameter("onesb", [128, 16], bf16, isOutput=False)
    d_mb = nc.declare_dram_parameter("mbcol", [128, 1], f32, isOutput=False)
    d_gate = nc.declare_dram_parameter("gatecol", [128, 1], f32, isOutput=False)
    d_idf = nc.declare_dram_parameter("idf", [128, 128], f32, isOutput=False)
    d_idb = nc.declare_dram_parameter("idb", [128, 128], bf16, isOutput=False)
    d_bq = nc.declare_dram_parameter("bqb", [128, 2], f32, isOutput=False)
    d_bk = nc.declare_dram_parameter("bkb", [128, 2], f32, isOutput=False)
    d_bv = nc.declare_dram_parameter("bvb", [128, 2], f32, isOutput=False)
    d_out = nc.declare_dram_parameter("out", [N // 4, C], f32, isOutput=True)
    d_dbg = nc.declare_dram_parameter("dbg", [128, 2112], f32, isOutput=True)

    from contextlib import ExitStack
    with tile.TileContext(nc) as tc:
        with (
            tc.tile_pool(name="cst", bufs=1) as cst,
            tc.tile_pool(name="qk", bufs=1) as qk,
            tc.tile_pool(name="ps_a", bufs=2, space="PSUM") as ps_a,
            tc.tile_pool(name="ps_cnt", bufs=1, space="PSUM") as ps_cnt,
            tc.tile_pool(name="ps_b", bufs=1, space="PSUM") as ps_b,
            tc.tile_pool(name="dram", bufs=1, space="DRAM") as dram,
        ):
            stage1 = ExitStack()
            qtp = stage1.enter_context(tc.tile_pool(name="qtp", bufs=1))
            tmp = stage1.enter_context(tc.tile_pool(name="tmp", bufs=3))
            # ---- constants in ----
            maskT = cst.tile([128, ST], f32)
            om = cst.tile([128, 2 * ST], bf16)
            onesb = cst.tile([128, 16], bf16)
            mbcol = cst.tile([128, 1], f32)
            gatecol = cst.tile([128, 1], f32)
            idf = cst.tile([128, 128], f32)
            idb = cst.tile([128, 128], bf16)
            bqb = cst.tile([128, 2], f32)
            bkb = cst.tile([128, 2], f32)
            bvb = cst.tile([128, 2], f32)
            for t, d in [(maskT, d_maskT), (om, d_om), (onesb, d_ones),
                         (mbcol, d_mb), (gatecol, d_gate), (idf, d_idf),
                         (idb, d_idb), (bqb, d_bq), (bkb, d_bk), (bvb, d_bv)]:
                nc.sync.dma_start(t[:], d[:])

            # ---- load + round queryT and weights to f32r ----
            qtr = qtp.tile([128, 8 * N], mybir.dt.float32r)   # 8 c-tiles stacked
            for i in range(8):
                t = tmp.tile([128, N], f32, tag="ld")
                nc.sync.dma_start(t[:], d_qt[i * 128:(i + 1) * 128, :])
                nc.vector.tensor_copy(qtr[:, i * N:(i + 1) * N], t[:])
            wtr = {}
            for nm, d_w in (("q", d_wq), ("k", d_wk), ("v", d_wv)):
                wr = qtp.tile([128, 8 * 2 * HD], mybir.dt.float32r, tag=f"w{nm}")
                for i in range(8):
                    t = tmp.tile([128, 2 * HD], f32, tag="ldw")
                    nc.sync.dma_start(t[:], d_w[i * 128:(i + 1) * 128, :])
                    nc.scalar.activation(wr[:, i * 2 * HD:(i + 1) * 2 * HD], t[:],
                                         AF.Copy, bias=0.0, scale=1.0)
                wtr[nm] = wr
            wpr = qtp.tile([128, 2 * C], mybir.dt.float32r)
            wpb = cst.tile([128, 2 * C], bf16)
            for h in range(2):
                t = tmp.tile([128, C], f32, tag="ld")
                nc.sync.dma_start(t[:], d_wp[h * 128:(h + 1) * 128, :])
                nc.scalar.activation(wpr[:, h * C:(h + 1) * C], t[:],
                                     AF.Copy, bias=0.0, scale=1.0)
                nc.vector.tensor_copy(wpb[:, h * C:(h + 1) * C], t[:])

            # ---- QKV projections (f32r), per head ----
            qT = [qk.tile([128, N], mybir.dt.float32r, tag=f"q{h}", name=f"qT{h}") for h in range(2)]
            kT = [qk.tile([128, N], mybir.dt.float32r, tag=f"k{h}", name=f"kT{h}") for h in range(2)]
            vTb = [qtp.tile([128, N], bf16, tag=f"v{h}", name=f"vTb{h}") for h in range(2)]
            for h in range(2):
                for nm, dst, bias_t, scale in (("q", qT[h], bqb, SCALE),
                                               ("k", kT[h], bkb, 1.0),
                                               ("v", vTb[h], bvb, 1.0)):
                    for ch in range(4):
                        ps = ps_a.tile([128, 512], f32, tag="a")
                        for ci in range(8):
                            lhs = wtr[nm][:, ci * 256 + h * 128: ci * 256 + (h + 1) * 128]
                            rhs = qtr[:, ci * N + ch * 512: ci * N + ch * 512 + 512]
                            nc.tensor.matmul(ps[:], lhs, rhs,
                                             start=(ci == 0), stop=(ci == 7))
                        nc.scalar.activation(dst[:, ch * 512:(ch + 1) * 512], ps[:],
                                             AF.Identity, bias=bias_t[:, h:h + 1],
                                             scale=scale)

            # ---- v transpose + mask; Vsum accumulation ----
            vm = [qk.tile([128, N], bf16, tag=f"vm{h}", name=f"vm{h}") for h in range(2)]
            vsum_r = []
            for h in range(2):
                for st in range(ST):
                    ps = ps_a.tile([128, 128], bf16, tag="a")
                    nc.tensor.transpose(ps[:], vTb[h][:, st * 128:(st + 1) * 128], idb[:])
                    nc.vector.tensor_scalar(vm[h][:, st * 128:(st + 1) * 128], ps[:],
                                            maskT[:, st:st + 1], None, OP.mult)
                pvs = ps_cnt.tile([128, 16], f32, tag="acc")
                for st in range(ST):
                    nc.tensor.matmul(pvs[:], vm[h][:, st * 128:(st + 1) * 128], onesb[:],
                                     start=(st == 0), stop=(st == ST - 1))
                vs = cst.tile([128, 1], mybir.dt.float32r, tag=f"vs{h}")
                nc.vector.tensor_copy(vs[:], pvs[:, 0:1])
                vsum_r.append(vs)
            # VsumP_h = Vsum_h @ Wp_h  -> [1, 1024] per head (r_n is per-head!)
            vsump = []
            for h in range(2):
                pvp = ps_b.tile([1, C], f32, tag="b")
                for ch in range(2):
                    nc.tensor.matmul(pvp[:, ch * 512:(ch + 1) * 512], vsum_r[h],
                                     wpr[:, h * C + ch * 512: h * C + ch * 512 + 512],
                                     start=True, stop=True)
                vp_h = cst.tile([1, C], bf16, tag=f"vsump{h}", name=f"vsump{h}")
                nc.vector.tensor_copy(vp_h[:], pvp[:])
                vsump.append(vp_h)
            vsumpf = vsump[0]  # debug alias

            stage1.close()
            stage2 = ExitStack()
            yb = stage2.enter_context(tc.tile_pool(name="yb", bufs=1))
            work = stage2.enter_context(tc.tile_pool(name="work", bufs=2))
            sres = stage2.enter_context(tc.tile_pool(name="sres", bufs=1))
            outp = stage2.enter_context(tc.tile_pool(name="outp", bufs=2))

            # ---- per head: topk, counts, Yv, normalize factors ----
            S_sb = [sres.tile([128, N], bf16, tag=f"s{h}", name=f"Ssb{h}") for h in range(2)]
            wcol = []   # w = r*em1 per head, [128, 16] fp32
            rgwT = []   # [16, 128] f32r per head
            for h in range(2):
                ybig = yb.tile([128, ST * N], bf16, tag="y")
                pyv = ps_cnt.tile([128, N], f32, tag="acc")
                for st in range(ST):
                    raw = work.tile([128, N], f32, tag="raw")
                    for ch in range(4):
                        ps = ps_a.tile([128, 512], f32, tag="a")
                        nc.tensor.matmul(ps[:], kT[h][:, st * 128:(st + 1) * 128],
                                         qT[h][:, ch * 512:(ch + 1) * 512],
                                         start=True, stop=True)
                        nc.scalar.activation(raw[:, ch * 512:(ch + 1) * 512], ps[:],
                                             AF.Copy, bias=0.0, scale=1.0)
                    top8 = work.tile([128, 8], f32, tag="top8")
                    nc.vector.max(top8[:], raw[:])
                    ytile = ybig[:, st * N:(st + 1) * N]
                    nc.vector.tensor_scalar(ytile, raw[:], top8[:, K - 1:K], None, OP.is_ge)
                    for ch in range(4):
                        nc.tensor.matmul(pyv[:, ch * 512:(ch + 1) * 512],
                                         vm[h][:, st * 128:(st + 1) * 128],
                                         ybig[:, st * N + ch * 512: st * N + ch * 512 + 512],
                                         start=(st == 0), stop=(st == ST - 1))
                for ch in range(4):
                    nc.scalar.activation(S_sb[h][:, ch * 512:(ch + 1) * 512],
                                         pyv[:, ch * 512:(ch + 1) * 512],
                                         AF.Copy, bias=0.0, scale=1.0)
                # counts matmuls (reuse the acc psum slot after S evac)
                pcnt = ps_cnt.tile([2, N], f32, tag="acc")
                for st in range(ST):
                    for ch in range(4):
                        nc.tensor.matmul(pcnt[:, ch * 512:(ch + 1) * 512],
                                         om[:, 2 * st:2 * st + 2],
                                         ybig[:, st * N + ch * 512: st * N + ch * 512 + 512],
                                         start=(st == 0), stop=(st == ST - 1))
                # counts -> [128, 32] via PE transpose
                cnt_sb = work.tile([2, N], f32, tag="cnt_sb", bufs=1)
                nc.vector.tensor_copy(cnt_sb[:], pcnt[:])
                ptr = ps_b.tile([128, 32], f32, tag="b")
                for t2 in range(ST):
                    nc.tensor.transpose(ptr[:, 2 * t2:2 * t2 + 2],
                                        cnt_sb[:, t2 * 128:(t2 + 1) * 128],
                                        idf[:2, :2])
                cntT = work.tile([128, 32], f32, tag="cntT")
                nc.vector.tensor_copy(cntT[:], ptr[:])
                cN = work.tile([128, 16], f32, tag="cN")
                cM = work.tile([128, 16], f32, tag="cM")
                nc.vector.tensor_copy(cN[:], cntT[:, 0:32:2])
                nc.vector.tensor_copy(cM[:], cntT[:, 1:32:2])
                rec = work.tile([128, 16], f32, tag="rec")
                nc.vector.tensor_scalar(rec[:], cN[:], 1.0, None, OP.add)
                nc.vector.reciprocal(rec[:], rec[:])
                e = work.tile([128, 16], f32, tag="e")
                nc.scalar.activation(e[:], rec[:], AF.Exp)
                em1 = work.tile([128, 16], f32, tag="em1")
                nc.vector.tensor_scalar(em1[:], e[:], -1.0, None, OP.add)
                Z = work.tile([128, 16], f32, tag="Z")
                nc.vector.tensor_mul(Z[:], em1[:], cM[:])
                nc.vector.tensor_scalar(Z[:], Z[:], mbcol[:, 0:1], None, OP.add)
                r_ = work.tile([128, 16], f32, tag="r_")
                nc.vector.reciprocal(r_[:], Z[:])
                w_ = work.tile([128, 16], f32, tag=f"w{h}_", name=f"w{h}_")
                nc.vector.tensor_mul(w_[:], r_[:], em1[:])
                wcol.append(w_)
                rem = work.tile([128, 16], f32, tag="rem")
                nc.vector.reciprocal(rem[:], em1[:])
                rgw = work.tile([128, 16], f32, tag="rgw")
                nc.vector.tensor_scalar(rgw[:], rem[:], gatecol[:, 0:1], None, OP.mult)
                prt = ps_b.tile([16, 128], f32, tag="b")
                nc.tensor.transpose(prt[:], rgw[:], idf[:])
                rgt16 = work.tile([16, 128], f32, tag="rgT16", bufs=1)
                nc.vector.tensor_copy(rgt16[:], prt[:])
                rgt1 = work.tile([1, 16 * 128], f32, tag="rgT1", bufs=1)
                nc.sync.dma_start(rgt1[:], rgt16[:])
                rgt = work.tile([1, 16 * 128], bf16, tag=f"rgTf{h}", name=f"rgTf{h}")
                nc.vector.tensor_copy(rgt[:], rgt1[:])
                rgwT.append(rgt)

            # w0 must be combined with w1: out = w0*S0@Wp0 + w1*S1@Wp1 + outer terms.
            # Since w differs per head, scale cannot be applied at a shared evac.
            # Instead: psum accumulates S0@Wp0*? -> need per-head scaling BEFORE the
            # matmul: scale S^T columns by w (free axis)... not possible. So:
            # evacuate per-head psums separately and add on DVE.
            partialA = dram.tile([N, 512], f32)
            partialB = dram.tile([N, 512], f32)
            rsA = dram.tile([N // 4, 512], f32)
            rsB = dram.tile([N // 4, 512], f32)
            for cch in range(2):
                partial_d = partialA if cch == 0 else partialB
                for nt in range(ST):
                    ob = outp.tile([128, 512], f32, tag="ob")
                    ps0 = ps_a.tile([128, 512], f32, tag="a")
                    nc.tensor.matmul(ps0[:], rgwT[0][0:1, nt * 128:(nt + 1) * 128],
                                     vsump[0][0:1, cch * 512:(cch + 1) * 512],
                                     start=True, stop=False)
                    nc.tensor.matmul(ps0[:], S_sb[0][:, nt * 128:(nt + 1) * 128],
                                     wpb[:, 0 * C + cch * 512: 0 * C + cch * 512 + 512],
                                     start=False, stop=True)
                    nc.scalar.activation(ob[:], ps0[:],
                                         AF.Copy, bias=0.0, scale=wcol[0][:, nt:nt + 1])
                    ps1 = ps_a.tile([128, 512], f32, tag="a")
                    nc.tensor.matmul(ps1[:], rgwT[1][0:1, nt * 128:(nt + 1) * 128],
                                     vsump[1][0:1, cch * 512:(cch + 1) * 512],
                                     start=True, stop=False)
                    nc.tensor.matmul(ps1[:], S_sb[1][:, nt * 128:(nt + 1) * 128],
                                     wpb[:, 1 * C + cch * 512: 1 * C + cch * 512 + 512],
                                     start=False, stop=True)
                    sc1 = outp.tile([128, 512], f32, tag="sc1")
                    nc.scalar.activation(sc1[:], ps1[:], AF.Copy, bias=0.0,
                                         scale=wcol[1][:, nt:nt + 1])
                    nc.vector.tensor_add(ob[:], ob[:], sc1[:])
                    nc.sync.dma_start(partial_d[nt * 128:(nt + 1) * 128, :], ob[:])
                rs_d = rsA if cch == 0 else rsB
                nc.gpsimd.collective_compute(
                    "ReduceScatter", OP.add,
                    replica_groups=[[0, 1, 2, 3], [4, 5, 6, 7]],
                    ins=[partial_d[:].opt()], outs=[rs_d[:].opt()])
                nc.gpsimd.dma_start(d_out[:, cch * 512:(cch + 1) * 512], rs_d[:])
            stage2.close()

    nc.compile()
    return nc


def _host_inputs(query, mask, Wq, bq, Wk, bk, Wv, bv, Wp, bp):
    """Per-core input dicts."""
    bf = ml_dtypes.bfloat16
    ins = []
    idf = np.eye(128, dtype=np.float32)
    idb = np.eye(128, dtype=bf)
    onesb = np.ones((128, 16), dtype=bf)
    for c in range(NCORES):
        b, g = c // 4, c % 4
        h0 = 2 * g
        qt = np.ascontiguousarray(query[b].T.astype(np.float32))
        sl = slice(h0 * HD, (h0 + 2) * HD)
        maskT = np.ascontiguousarray(
            mask[b].reshape(ST, 128).T.astype(np.float32))
        om = np.zeros((128, 2 * ST), dtype=bf)
        om[:, 0::2] = 1.0
        om[:, 1::2] = maskT.astype(bf)
        mbcol = np.full((128, 1), float(mask[b].sum()), dtype=np.float32)
        gatecol = np.full((128, 1), 1.0, dtype=np.float32)
        bqb = np.stack([SCALE * bq[(h0 + i) * HD:(h0 + i + 1) * HD] for i in range(2)],
                       axis=1).astype(np.float32)
        bkb = np.stack([bk[(h0 + i) * HD:(h0 + i + 1) * HD] for i in range(2)],
                       axis=1).astype(np.float32)
        bvb = np.stack([bv[(h0 + i) * HD:(h0 + i + 1) * HD] for i in range(2)],
                       axis=1).astype(np.float32)
        ins.append(dict(
            qt=qt,
            wq=np.ascontiguousarray(Wq[:, sl].astype(np.float32)),
            wk=np.ascontiguousarray(Wk[:, sl].astype(np.float32)),
            wv=np.ascontiguousarray(Wv[:, sl].astype(np.float32)),
            wp=np.ascontiguousarray(Wp[sl, :].astype(np.float32)),
            maskT=maskT, om=om, onesb=onesb, mbcol=mbcol, gatecol=gatecol,
            idf=idf, idb=idb, bqb=bqb, bkb=bkb, bvb=bvb))
    return ins


def kernel(query, mask, Wq, bq, Wk, bk, Wv, bv, Wp, bp):
    from concourse.bass_utils import run_bass_kernel_spmd

    if "nc" not in _cache:
        _cache["nc"] = _build()
    nc = _cache["nc"]
    ins = _host_inputs(query, mask, Wq, bq, Wk, bk, Wv, bv, Wp, bp)
    res = run_bass_kernel_spmd(nc, ins, list(range(NCORES)))
    out = np.empty((B, N, C), dtype=np.float32)
    for b in range(B):
        out[b] = np.concatenate(
            [res.results[4 * b + p]["out"] for p in range(4)], axis=0)
    out += np.asarray(bp, dtype=np.float32)[None, None, :]
    return out



# revision 7
# speedup vs baseline: 1.4102x; 1.4102x over previous
"""AssignAttention (topk_masking) Trainium2 kernel — 8 NeuronCores.

Sharding: data-parallel over B (2 groups of 4 cores), tensor-parallel over
heads H (2 heads per core). Per core: QKV projections for its 2 heads (fp16
matmuls, f32 PSUM accumulation), rawT = k q^T per head, per-column top-4 via
the DVE MAX8 instruction on an fp16 copy, binary Y^T mask, count matmuls,
sparse-softmax reformulation (exp(A) = 1 + (e_n-1) Y), Yv matmul, output
projection with fused normalization, fp16 ReduceScatter(add) over each
4-core group.

Math: with Y[n,s] = [n in top4 of column s], c_n = sum_s Y, cm_n = sum_s
mask_s Y, e_n = exp(1/(c_n+1)), M = sum_s mask_s, Z_n = M + (e_n-1) cm_n:
out_head[n,:] = (Vsum + (e_n-1) * (Y.mask @ v)[n,:]) / Z_n
which equals the reference's hard-topk + sum-normalize + masked softmax.
"""
import sys, os

os.environ["JAX_ENABLE_COMPILATION_CACHE"] = "false"
sys.path.insert(0, "/opt/trn_rl_repo")
import numpy as np
import ml_dtypes

B, N, C, H, K = 2, 2048, 1024, 8, 4
HD = C // H
SCALE = HD ** -0.5
NCORES = 8
ST = 16          # s-tiles per head

_cache = {}


def _build():
    from concourse import bacc, tile, mybir

    f32, f16 = mybir.dt.float32, mybir.dt.float16
    AF = mybir.ActivationFunctionType
    OP = mybir.AluOpType

    nc = bacc.Bacc(None, target_bir_lowering=False)
    d_qt = nc.declare_dram_parameter("qt", [C, N], f16, isOutput=False)
    d_wq = nc.declare_dram_parameter("wq", [C, 2 * HD], f16, isOutput=False)
    d_wk = nc.declare_dram_parameter("wk", [C, 2 * HD], f16, isOutput=False)
    d_wv = nc.declare_dram_parameter("wv", [C, 2 * HD], f16, isOutput=False)
    d_wp = nc.declare_dram_parameter("wp", [2 * HD, C], f16, isOutput=False)
    d_maskT = nc.declare_dram_parameter("maskT", [128, ST], f32, isOutput=False)
    d_om = nc.declare_dram_parameter("om", [128, 2 * ST], f16, isOutput=False)
    d_ones = nc.declare_dram_parameter("onesc", [128, 16], f16, isOutput=False)
    d_mb = nc.declare_dram_parameter("mbcol", [128, 1], f32, isOutput=False)
    d_idt = nc.declare_dram_parameter("idt", [128, 128], f16, isOutput=False)
    d_idf = nc.declare_dram_parameter("idf", [128, 128], f32, isOutput=False)
    d_bqkv = nc.declare_dram_parameter("bqkv", [128, 6], f32, isOutput=False)
    d_out = nc.declare_dram_parameter("out", [N // 4, C], f16, isOutput=True)

    from contextlib import ExitStack
    with tile.TileContext(nc) as tc:
        with (
            tc.tile_pool(name="cst", bufs=1) as cst,
            tc.tile_pool(name="proj", bufs=1) as proj,
            tc.tile_pool(name="dram", bufs=1, space="DRAM") as dram,
        ):
            # ---- constants in ----
            maskT = cst.tile([128, ST], f32)
            om = cst.tile([128, 2 * ST], f16)
            onesc = cst.tile([128, 16], f16)
            mbcol = cst.tile([128, 1], f32)
            idt = cst.tile([128, 128], f16)
            idf = cst.tile([128, 128], f32)
            bqkv = cst.tile([128, 6], f32)
            for t, d in [(maskT, d_maskT), (om, d_om), (onesc, d_ones),
                         (mbcol, d_mb), (idt, d_idt), (idf, d_idf),
                         (bqkv, d_bqkv)]:
                nc.sync.dma_start(t[:], d[:])
            wp_sb = cst.tile([128, 2 * C], f16)
            for h in range(2):
                nc.sync.dma_start(wp_sb[:, h * C:(h + 1) * C],
                                  d_wp[h * 128:(h + 1) * 128, :])

            stage1 = ExitStack()
            ld = stage1.enter_context(tc.tile_pool(name="ld", bufs=1))
            ps_a = stage1.enter_context(
                tc.tile_pool(name="ps_a", bufs=2, space="PSUM"))
            # ---- load fp16 query^T and weights ----
            qt_sb = ld.tile([128, 8 * N], f16)
            for i in range(8):
                nc.sync.dma_start(qt_sb[:, i * N:(i + 1) * N],
                                  d_qt[i * 128:(i + 1) * 128, :])
            w_sb = {}
            for nm, d_w in (("q", d_wq), ("k", d_wk), ("v", d_wv)):
                w = ld.tile([128, 8 * 2 * HD], f16, tag=f"w{nm}")
                for i in range(8):
                    nc.sync.dma_start(w[:, i * 256:(i + 1) * 256],
                                      d_w[i * 128:(i + 1) * 128, :])
                w_sb[nm] = w

            # ---- QKV projections (fp16), per head; out layout [hd, n] ----
            qT = [proj.tile([128, N], f16, tag=f"q{h}", name=f"qT{h}") for h in range(2)]
            kT = [proj.tile([128, N], f16, tag=f"k{h}", name=f"kT{h}") for h in range(2)]
            vTb = [ld.tile([128, N], f16, tag=f"v{h}", name=f"vTb{h}") for h in range(2)]
            for h in range(2):
                for bi, (nm, dst) in enumerate((("q", qT[h]), ("k", kT[h]),
                                                ("v", vTb[h]))):
                    for ch in range(4):
                        ps = ps_a.tile([128, 512], f32, tag="a")
                        for ci in range(8):
                            lhs = w_sb[nm][:, ci * 256 + h * 128:
                                           ci * 256 + (h + 1) * 128]
                            rhs = qt_sb[:, ci * N + ch * 512:
                                        ci * N + ch * 512 + 512]
                            nc.tensor.matmul(ps[:], lhs, rhs,
                                             start=(ci == 0), stop=(ci == 7))
                        nc.scalar.activation(dst[:, ch * 512:(ch + 1) * 512],
                                             ps[:], AF.Identity,
                                             bias=bqkv[:, 2 * bi + h:2 * bi + h + 1],
                                             scale=1.0)

            # ---- v transpose + mask -> vm [s, hd]; Vsum; vsump ----
            vm = [proj.tile([128, N], f16, tag=f"vm{h}", name=f"vm{h}") for h in range(2)]
            vsump = [cst.tile([1, C], f16, tag=f"vsump{h}", name=f"vsump{h}")
                     for h in range(2)]        # per-head Vsum @ Wp row
            for h in range(2):
                for st in range(ST):
                    ps = ps_a.tile([128, 128], f16, tag="a")
                    nc.tensor.transpose(ps[:], vTb[h][:, st * 128:(st + 1) * 128],
                                        idt[:])
                    nc.vector.tensor_scalar(vm[h][:, st * 128:(st + 1) * 128],
                                            ps[:], maskT[:, st:st + 1], None,
                                            OP.mult)
                pvs = ps_a.tile([128, 16], f32, tag="a")
                for st in range(ST):
                    nc.tensor.matmul(pvs[:], vm[h][:, st * 128:(st + 1) * 128],
                                     onesc[:], start=(st == 0), stop=(st == ST - 1))
                vs = cst.tile([128, 1], f16, tag=f"vs{h}", name=f"vs{h}")
                nc.vector.tensor_copy(vs[:], pvs[:, 0:1])
                pvp = ps_a.tile([1, C], f32, tag="vp", bufs=1)
                for ch in range(2):
                    nc.tensor.matmul(pvp[:, ch * 512:(ch + 1) * 512], vs[:],
                                     wp_sb[:, h * C + ch * 512:h * C + ch * 512 + 512],
                                     start=True, stop=True)
                nc.vector.tensor_copy(vsump[h][:], pvp[:])
            stage1.close()

            stage2 = ExitStack()
            yb = stage2.enter_context(tc.tile_pool(name="yb", bufs=1))
            work = stage2.enter_context(tc.tile_pool(name="work", bufs=2))
            outp = stage2.enter_context(tc.tile_pool(name="outp", bufs=2))
            ps_raw = stage2.enter_context(
                tc.tile_pool(name="ps_raw", bufs=4, space="PSUM"))
            ps_yv = stage2.enter_context(
                tc.tile_pool(name="ps_yv", bufs=1, space="PSUM"))
            ps_cnt = stage2.enter_context(
                tc.tile_pool(name="ps_cnt", bufs=1, space="PSUM"))

            S_sb = [proj.tile([128, N], f16, tag=f"s{h}", name=f"Ssb{h}") for h in range(2)]
            wcol = []   # w = em1/Z per head, [128, 16] f32 (col = n-tile)
            rgwT = []   # 1/em1 as row [1, N] f16 per head
            for h in range(2):
                ybig = yb.tile([128, ST * N], f16, tag=f"y{h}")
                # phase A: raw scores -> fp16 -> top4 threshold -> Y
                for st in range(ST):
                    rawf = work.tile([128, N], f16, tag="raw")
                    for ch in range(4):
                        ps = ps_raw.tile([128, 512], f32, tag="r")
                        nc.tensor.matmul(ps[:], kT[h][:, st * 128:(st + 1) * 128],
                                         qT[h][:, ch * 512:(ch + 1) * 512],
                                         start=True, stop=True)
                        nc.scalar.activation(rawf[:, ch * 512:(ch + 1) * 512],
                                             ps[:], AF.Copy, bias=0.0, scale=1.0)
                    top8 = work.tile([128, 8], f32, tag="top8")
                    nc.vector.max(top8[:], rawf[:])
                    nc.vector.tensor_scalar(ybig[:, st * N:(st + 1) * N],
                                            rawf[:], top8[:, K - 1:K], None,
                                            OP.is_ge)
                # phase B1: counts (c_n, cm_n) via om^T @ Y
                cnt_sb = work.tile([2, N], f16, tag="cnt", bufs=1)
                for ch in range(4):
                    pc = ps_cnt.tile([2, 512], f32, tag="c")
                    for st in range(ST):
                        nc.tensor.matmul(pc[:], om[:, 2 * st:2 * st + 2],
                                         ybig[:, st * N + ch * 512:
                                              st * N + ch * 512 + 512],
                                         start=(st == 0), stop=(st == ST - 1))
                    nc.vector.tensor_copy(cnt_sb[:, ch * 512:(ch + 1) * 512],
                                          pc[:])
                ptr = ps_cnt.tile([128, 32], f16, tag="tr")
                for t2 in range(ST):
                    nc.tensor.transpose(ptr[:, 2 * t2:2 * t2 + 2],
                                        cnt_sb[:, t2 * 128:(t2 + 1) * 128],
                                        idt[:2, :2])
                cntT = work.tile([128, 32], f32, tag="cntT")
                nc.vector.tensor_copy(cntT[:], ptr[:])
                rec = work.tile([128, 16], f32, tag="rec")
                nc.vector.tensor_scalar(rec[:], cntT[:, 0:32:2], 1.0, None, OP.add)
                nc.vector.reciprocal(rec[:], rec[:])
                e = work.tile([128, 16], f32, tag="e")
                nc.scalar.activation(e[:], rec[:], AF.Exp)
                em1 = work.tile([128, 16], f32, tag="em1")
                nc.vector.tensor_scalar(em1[:], e[:], -1.0, None, OP.add)
                Z = work.tile([128, 16], f32, tag="Z")
                nc.vector.tensor_mul(Z[:], em1[:], cntT[:, 1:32:2])
                nc.vector.tensor_scalar(Z[:], Z[:], mbcol[:, 0:1], None, OP.add)
                r_ = work.tile([128, 16], f32, tag="r_")
                nc.vector.reciprocal(r_[:], Z[:])
                w_ = work.tile([128, 16], f32, tag=f"w{h}_", name=f"w{h}_")
                nc.vector.tensor_mul(w_[:], r_[:], em1[:])
                wcol.append(w_)
                rem = work.tile([128, 16], f32, tag="rem")
                nc.vector.reciprocal(rem[:], em1[:])
                prt = ps_cnt.tile([16, 128], f32, tag="tr")
                nc.tensor.transpose(prt[:], rem[:], idf[:])
                rgt16 = work.tile([16, 128], f16, tag="rgT16", bufs=1)
                nc.vector.tensor_copy(rgt16[:], prt[:])
                rgt = work.tile([1, 16 * 128], f16, tag=f"rgTf{h}", name=f"rgTf{h}")
                nc.sync.dma_start(rgt[:], rgt16[:])
                rgwT.append(rgt)
                # phase B2: S = vm^T @ Y  (accumulate over s-tiles per n-chunk)
                for ch in range(4):
                    py = ps_yv.tile([128, 512], f32, tag="yv")
                    for st in range(ST):
                        nc.tensor.matmul(py[:], vm[h][:, st * 128:(st + 1) * 128],
                                         ybig[:, st * N + ch * 512:
                                              st * N + ch * 512 + 512],
                                         start=(st == 0), stop=(st == ST - 1))
                    nc.scalar.activation(S_sb[h][:, ch * 512:(ch + 1) * 512],
                                         py[:], AF.Copy, bias=0.0, scale=1.0)

            # ---- output: per head psum (rank-1 Vsum/Z term + S@Wp), scale,
            #      add, DMA partials, fp16 ReduceScatter over the group ----
            partialA = dram.tile([N, 512], f16)
            partialB = dram.tile([N, 512], f16)
            rsA = dram.tile([N // 4, 512], f16)
            rsB = dram.tile([N // 4, 512], f16)
            for cch in range(2):
                partial_d = partialA if cch == 0 else partialB
                for nt in range(ST):
                    ob = outp.tile([128, 512], f16, tag="ob")
                    ps0 = ps_raw.tile([128, 512], f32, tag="r")
                    nc.tensor.matmul(ps0[:], rgwT[0][0:1, nt * 128:(nt + 1) * 128],
                                     vsump[0][0:1, cch * 512:(cch + 1) * 512],
                                     start=True, stop=False)
                    nc.tensor.matmul(ps0[:], S_sb[0][:, nt * 128:(nt + 1) * 128],
                                     wp_sb[:, 0 * C + cch * 512:0 * C + cch * 512 + 512],
                                     start=False, stop=True)
                    nc.scalar.activation(ob[:], ps0[:], AF.Copy, bias=0.0,
                                         scale=wcol[0][:, nt:nt + 1])
                    ps1 = ps_raw.tile([128, 512], f32, tag="r")
                    nc.tensor.matmul(ps1[:], rgwT[1][0:1, nt * 128:(nt + 1) * 128],
                                     vsump[1][0:1, cch * 512:(cch + 1) * 512],
                                     start=True, stop=False)
                    nc.tensor.matmul(ps1[:], S_sb[1][:, nt * 128:(nt + 1) * 128],
                                     wp_sb[:, 1 * C + cch * 512:1 * C + cch * 512 + 512],
                                     start=False, stop=True)
                    sc1 = outp.tile([128, 512], f16, tag="sc1")
                    nc.scalar.activation(sc1[:], ps1[:], AF.Copy, bias=0.0,
                                         scale=wcol[1][:, nt:nt + 1])
                    nc.vector.tensor_add(ob[:], ob[:], sc1[:])
                    nc.sync.dma_start(partial_d[nt * 128:(nt + 1) * 128, :], ob[:])
                rs_d = rsA if cch == 0 else rsB
                OPa = OP.add
                nc.gpsimd.collective_compute(
                    "ReduceScatter", OPa,
                    replica_groups=[[0, 1, 2, 3], [4, 5, 6, 7]],
                    ins=[partial_d[:].opt()], outs=[rs_d[:].opt()])
                nc.gpsimd.dma_start(d_out[:, cch * 512:(cch + 1) * 512], rs_d[:])
            stage2.close()

    nc.compile()
    return nc


def _host_inputs(query, mask, Wq, bq, Wk, bk, Wv, bv, Wp, bp):
    """Per-core input dicts (fp16 activations/weights, SCALE folded into Wq)."""
    f16 = np.float16
    ins = []
    idt = np.eye(128, dtype=f16)
    idf = np.eye(128, dtype=np.float32)
    onesc = np.ones((128, 16), dtype=f16)
    for c in range(NCORES):
        b, g = c // 4, c % 4
        h0 = 2 * g
        sl = slice(h0 * HD, (h0 + 2) * HD)
        qt = np.ascontiguousarray(query[b].T).astype(f16)
        maskT = np.ascontiguousarray(
            mask[b].reshape(ST, 128).T.astype(np.float32))
        om = np.zeros((128, 2 * ST), dtype=f16)
        om[:, 0::2] = 1.0
        om[:, 1::2] = maskT.astype(f16)
        mbcol = np.full((128, 1), float(mask[b].sum()), dtype=np.float32)
        bqkv = np.zeros((128, 6), dtype=np.float32)
        for i in range(2):
            bqkv[:, 0 + i] = SCALE * bq[(h0 + i) * HD:(h0 + i + 1) * HD]
            bqkv[:, 2 + i] = bk[(h0 + i) * HD:(h0 + i + 1) * HD]
            bqkv[:, 4 + i] = bv[(h0 + i) * HD:(h0 + i + 1) * HD]
        ins.append(dict(
            qt=qt,
            wq=np.ascontiguousarray(Wq[:, sl] * SCALE).astype(f16),
            wk=np.ascontiguousarray(Wk[:, sl]).astype(f16),
            wv=np.ascontiguousarray(Wv[:, sl]).astype(f16),
            wp=np.ascontiguousarray(Wp[sl, :]).astype(f16),
            maskT=maskT, om=om, onesc=onesc, mbcol=mbcol,
            idt=idt, idf=idf, bqkv=bqkv))
    return ins


def kernel(query, mask, Wq, bq, Wk, bk, Wv, bv, Wp, bp):
    from concourse.bass_utils import run_bass_kernel_spmd

    if "nc" not in _cache:
        _cache["nc"] = _build()
    nc = _cache["nc"]
    ins = _host_inputs(query, mask, Wq, bq, Wk, bk, Wv, bv, Wp, bp)
    res = run_bass_kernel_spmd(nc, ins, list(range(NCORES)))
    out = np.empty((B, N, C), dtype=np.float32)
    for b in range(B):
        out[b] = np.concatenate(
            [res.results[4 * b + p]["out"].astype(np.float32)
             for p in range(4)], axis=0)
    out += np.asarray(bp, dtype=np.float32)[None, None, :]
    return out


# revision 11
# speedup vs baseline: 1.5209x; 1.0785x over previous
"""AssignAttention (topk_masking) Trainium2 kernel — 8 NeuronCores.

Sharding: data-parallel over B (2 groups of 4 cores), tensor-parallel over
heads H (2 heads per core). fp16 matmuls with f32 PSUM accumulation; Y and
the Y-consuming matmuls (counts, Yv) run in fp8e4 with DoubleRow perf mode.
Emission is software-pipelined: the QKV projection groups are interleaved
into head-0's raw/topk loop and head-0's counts/Yv matmuls into head-1's
raw/topk loop so Tensor/Scalar/Vector stay concurrently busy. The final
ReduceScatter is sliced 4x so it starts while output tiles are still being
computed.

Math: with Y[n,s] = [n in top4 of column s], c_n = sum_s Y, cm_n = sum_s
mask_s Y, e_n = exp(1/(c_n+1)), M = sum_s mask_s, Z_n = M + (e_n-1) cm_n:
out_head[n,:] = (Vsum + (e_n-1) * (Y.mask @ v)[n,:]) / Z_n
which equals the reference's hard-topk + sum-normalize + masked softmax.
"""
import sys, os

os.environ["JAX_ENABLE_COMPILATION_CACHE"] = "false"
sys.path.insert(0, "/opt/trn_rl_repo")
import numpy as np
import ml_dtypes

B, N, C, H, K = 2, 2048, 1024, 8, 4
HD = C // H
SCALE = HD ** -0.5
NCORES = 8
ST = 16          # s-tiles per head

_cache = {}


def _build():
    from concourse import bacc, tile, mybir

    f32, f16 = mybir.dt.float32, mybir.dt.float16
    f8 = mybir.dt.float8e4
    DR = mybir.MatmulPerfMode.DoubleRow
    AF = mybir.ActivationFunctionType
    OP = mybir.AluOpType

    nc = bacc.Bacc(None, target_bir_lowering=False)
    d_qt = nc.declare_dram_parameter("qt", [C, N], f16, isOutput=False)
    d_wq = nc.declare_dram_parameter("wq", [C, 2 * HD], f16, isOutput=False)
    d_wk = nc.declare_dram_parameter("wk", [C, 2 * HD], f16, isOutput=False)
    d_wv = nc.declare_dram_parameter("wv", [C, 2 * HD], f16, isOutput=False)
    d_wp = nc.declare_dram_parameter("wp", [2 * HD, C], f16, isOutput=False)
    d_maskT = nc.declare_dram_parameter("maskT", [128, ST], f32, isOutput=False)
    d_om = nc.declare_dram_parameter("om8", [128, ST, 2], f8, isOutput=False)
    d_ones = nc.declare_dram_parameter("onesc", [128, 16], f16, isOutput=False)
    d_mb = nc.declare_dram_parameter("mbcol", [128, 1], f32, isOutput=False)
    d_idt = nc.declare_dram_parameter("idt", [128, 128], f16, isOutput=False)
    d_idf = nc.declare_dram_parameter("idf", [128, 128], f32, isOutput=False)
    d_bqkv = nc.declare_dram_parameter("bqkv", [128, 6], f32, isOutput=False)
    d_out = nc.declare_dram_parameter("out", [N // 4, C], f16, isOutput=True)

    with tile.TileContext(nc) as tc:
        with (
            tc.tile_pool(name="cst", bufs=1) as cst,
            tc.tile_pool(name="proj", bufs=1) as proj,
            tc.tile_pool(name="ld", bufs=1) as ld,
            tc.tile_pool(name="yb", bufs=1) as yb,
            tc.tile_pool(name="work", bufs=2) as work,
            tc.tile_pool(name="outp", bufs=2) as outp,
            tc.tile_pool(name="ps_raw", bufs=5, space="PSUM") as ps_raw,
            tc.tile_pool(name="ps_yv", bufs=1, space="PSUM") as ps_yv,
            tc.tile_pool(name="ps_cnt", bufs=1, space="PSUM") as ps_cnt,
            tc.tile_pool(name="dram", bufs=1, space="DRAM") as dram,
        ):
            # ---- constants + inputs (DMAs spread over engine queues) ----
            maskT = cst.tile([128, ST], f32)
            om8 = cst.tile([128, ST, 2], f8)
            onesc = cst.tile([128, 16], f16)
            mbcol = cst.tile([128, 1], f32)
            idt = cst.tile([128, 128], f16)
            idf = cst.tile([128, 128], f32)
            bqkv = cst.tile([128, 6], f32)
            for t, d in [(maskT, d_maskT), (om8, d_om), (onesc, d_ones),
                         (mbcol, d_mb), (idt, d_idt), (idf, d_idf),
                         (bqkv, d_bqkv)]:
                nc.gpsimd.dma_start(t[:], d[:])
            wp_sb = cst.tile([128, 2 * C], f16)
            for h in range(2):
                nc.gpsimd.dma_start(wp_sb[:, h * C:(h + 1) * C],
                                    d_wp[h * 128:(h + 1) * 128, :])
            qt_sb = ld.tile([128, 8 * N], f16)
            w_sb = {nm: ld.tile([128, 8 * 2 * HD], f16, tag=f"w{nm}",
                                name=f"wsb{nm}")
                    for nm in ("q", "k", "v")}
            qs = [nc.sync, nc.scalar, nc.gpsimd]
            for i in range(8):
                nc.sync.dma_start(w_sb["q"][:, i * 256:(i + 1) * 256],
                                  d_wq[i * 128:(i + 1) * 128, :])
                nc.scalar.dma_start(w_sb["k"][:, i * 256:(i + 1) * 256],
                                    d_wk[i * 128:(i + 1) * 128, :])
                qs[i % 3].dma_start(qt_sb[:, i * N:(i + 1) * N],
                                    d_qt[i * 128:(i + 1) * 128, :])
            for i in range(8):
                nc.gpsimd.dma_start(w_sb["v"][:, i * 256:(i + 1) * 256],
                                    d_wv[i * 128:(i + 1) * 128, :])

            qT = [proj.tile([128, N], f16, tag=f"q{h}", name=f"qT{h}")
                  for h in range(2)]
            kT = [proj.tile([128, N], f16, tag=f"k{h}", name=f"kT{h}")
                  for h in range(2)]
            vTb = [ld.tile([128, N], f16, tag=f"v{h}", name=f"vTb{h}")
                   for h in range(2)]
            vm = [proj.tile([128, N], f16, tag=f"vm{h}", name=f"vm{h}")
                  for h in range(2)]
            vm8 = [proj.tile([128, ST, 128], f8, tag=f"vm8{h}", name=f"vm8{h}")
                   for h in range(2)]
            vsump = [cst.tile([1, C], f16, tag=f"vsump{h}", name=f"vsump{h}")
                     for h in range(2)]
            S_sb = [proj.tile([128, N], f16, tag=f"s{h}", name=f"Ssb{h}")
                    for h in range(2)]
            ybig8 = [yb.tile([128, ST, N], f8, tag=f"y{h}", name=f"ybig{h}")
                     for h in range(2)]

            BI = {"q": 0, "k": 1, "v": 2}

            def proj_group(nm, h, ch):
                dst = {"q": qT, "k": kT, "v": vTb}[nm][h]
                ps = ps_raw.tile([128, 512], f32, tag="r", name="pg")
                for ci in range(8):
                    nc.tensor.matmul(
                        ps[:],
                        w_sb[nm][:, ci * 256 + h * 128:ci * 256 + (h + 1) * 128],
                        qt_sb[:, ci * N + ch * 512:ci * N + ch * 512 + 512],
                        start=(ci == 0), stop=(ci == 7))
                nc.scalar.activation(dst[:, ch * 512:(ch + 1) * 512], ps[:],
                                     AF.Identity,
                                     bias=bqkv[:, 2 * BI[nm] + h:2 * BI[nm] + h + 1],
                                     scale=1.0)

            def vm_transpose(h, st):
                ps = ps_cnt.tile([128, 128], f16, tag="tr", name="vt")
                nc.tensor.transpose(ps[:], vTb[h][:, st * 128:(st + 1) * 128],
                                    idt[:])
                nc.vector.tensor_scalar(vm[h][:, st * 128:(st + 1) * 128],
                                        ps[:], maskT[:, st:st + 1], None,
                                        OP.mult)

            def vsum_calc(h):
                pvs = ps_cnt.tile([128, 16], f32, tag="tr", name="pvs")
                for st in range(ST):
                    nc.tensor.matmul(pvs[:], vm[h][:, st * 128:(st + 1) * 128],
                                     onesc[:], start=(st == 0),
                                     stop=(st == ST - 1))
                vs = cst.tile([128, 1], f16, tag=f"vs{h}", name=f"vs{h}")
                nc.vector.tensor_copy(vs[:], pvs[:, 0:1])
                return vs

            def vsump_calc(h, vs):
                for ch in range(2):
                    pvp = ps_cnt.tile([1, 512], f32, tag="tr", name="pvp")
                    nc.tensor.matmul(pvp[:], vs[:],
                                     wp_sb[:, h * C + ch * 512:
                                           h * C + ch * 512 + 512],
                                     start=True, stop=True)
                    nc.vector.tensor_copy(vsump[h][0:1, ch * 512:(ch + 1) * 512],
                                          pvp[:])

            def raw_step(h, st):
                rawf = work.tile([128, N], f16, tag="raw", name="rawf")
                for ch in range(4):
                    ps = ps_raw.tile([128, 512], f32, tag="r", name="psr")
                    nc.tensor.matmul(ps[:], kT[h][:, st * 128:(st + 1) * 128],
                                     qT[h][:, ch * 512:(ch + 1) * 512],
                                     start=True, stop=True)
                    nc.scalar.activation(rawf[:, ch * 512:(ch + 1) * 512],
                                         ps[:], AF.Copy, bias=0.0, scale=1.0)
                top8 = work.tile([128, 8], f32, tag="top8", name="top8")
                nc.vector.max(top8[:], rawf[:])
                nc.vector.tensor_scalar(ybig8[h][:, st, :], rawf[:],
                                        top8[:, K - 1:K], None, OP.is_ge)

            cnt_ps = {}

            def counts_pair(h, ch, sp):
                # accumulate om8^T @ Y over s-tile pair sp (DoubleRow fp8)
                if sp == 0:
                    cnt_ps[h] = ps_cnt.tile([2, 512], f32, tag="c", name="pc")
                for t in (2 * sp, 2 * sp + 1):
                    nc.tensor.matmul(cnt_ps[h][:], om8[:, t, :],
                                     ybig8[h][:, t, ch * 512:ch * 512 + 512],
                                     start=(t == 0), stop=(t == 2 * ST - 1))

            def counts_evac(h, ch, cnt_sb):
                nc.vector.tensor_copy(cnt_sb[:, ch * 512:(ch + 1) * 512],
                                      cnt_ps[h][:])

            yv_ps = {}

            def yv_pair(h, ch, sp):
                if sp == 0:
                    yv_ps[h] = ps_yv.tile([128, 512], f32, tag="yv", name="py")
                nc.tensor.matmul(yv_ps[h][:], vm8[h][:, 2 * sp:2 * sp + 2, :],
                                 ybig8[h][:, 2 * sp:2 * sp + 2,
                                          ch * 512:ch * 512 + 512],
                                 start=(sp == 0), stop=(sp == 7), perf_mode=DR)

            def yv_evac(h, ch):
                nc.scalar.activation(S_sb[h][:, ch * 512:(ch + 1) * 512],
                                     yv_ps[h][:], AF.Copy, bias=0.0, scale=1.0)

            def w_math(h, cnt_sb):
                ptr = ps_cnt.tile([128, 32], f16, tag="tr", name="ptr")
                for t2 in range(ST):
                    nc.tensor.transpose(ptr[:, 2 * t2:2 * t2 + 2],
                                        cnt_sb[:, t2 * 128:(t2 + 1) * 128],
                                        idt[:2, :2])
                cntT = work.tile([128, 32], f32, tag="cntT", name="cntT")
                nc.vector.tensor_copy(cntT[:], ptr[:])
                rec = work.tile([128, 16], f32, tag="rec", name="rec")
                nc.vector.tensor_scalar(rec[:], cntT[:, 0:32:2], 1.0, None,
                                        OP.add)
                nc.vector.reciprocal(rec[:], rec[:])
                e = work.tile([128, 16], f32, tag="e", name="e")
                nc.scalar.activation(e[:], rec[:], AF.Exp)
                em1 = work.tile([128, 16], f32, tag="em1", name="em1")
                nc.vector.tensor_scalar(em1[:], e[:], -1.0, None, OP.add)
                Z = work.tile([128, 16], f32, tag="Z", name="Zt")
                nc.vector.tensor_mul(Z[:], em1[:], cntT[:, 1:32:2])
                nc.vector.tensor_scalar(Z[:], Z[:], mbcol[:, 0:1], None, OP.add)
                r_ = work.tile([128, 16], f32, tag="r_", name="r_")
                nc.vector.reciprocal(r_[:], Z[:])
                w_ = work.tile([128, 16], f32, tag=f"w{h}_", name=f"w{h}_")
                nc.vector.tensor_mul(w_[:], r_[:], em1[:])
                rem = work.tile([128, 16], f32, tag="rem", name="rem")
                nc.vector.reciprocal(rem[:], em1[:])
                prt = ps_cnt.tile([16, 128], f32, tag="tr", name="prt")
                nc.tensor.transpose(prt[:], rem[:], idf[:])
                rgt16 = work.tile([16, 128], f16, tag="rgT16", bufs=1,
                                  name="rgt16")
                nc.vector.tensor_copy(rgt16[:], prt[:])
                rgt = work.tile([1, 16 * 128], f16, tag=f"rgTf{h}",
                                name=f"rgTf{h}")
                nc.sync.dma_start(rgt[:], rgt16[:])
                return w_, rgt

            # ================= schedule =================
            # pre: q0 + k0 projection groups
            for ch in range(4):
                proj_group("q", 0, ch)
            for ch in range(4):
                proj_group("k", 0, ch)

            # A0: head-0 raw/topk, interleaved with remaining projections
            PG = ([("v", 0, ch) for ch in range(4)] +
                  [("q", 1, ch) for ch in range(4)] +
                  [("k", 1, ch) for ch in range(4)] +
                  [("v", 1, ch) for ch in range(4)])
            vs0 = None
            for st in range(ST):
                proj_group(*PG[st])
                if 4 <= st < 12:
                    vm_transpose(0, 2 * (st - 4))
                    vm_transpose(0, 2 * (st - 4) + 1)
                if st == 12:
                    vs0 = vsum_calc(0)
                if st == 13:
                    vsump_calc(0, vs0)
                if st == 14:
                    nc.vector.tensor_copy(vm8[0][:], vm[0][:])
                raw_step(0, st)

            # A1: head-1 raw/topk, interleaved with head-0 counts + Yv
            cnt0 = work.tile([2, N], f16, tag="cnt", bufs=1, name="cnt0")
            w0 = rgw0 = None
            vs1 = None
            for st in range(ST):
                if st < 8:
                    vm_transpose(1, 2 * st)
                    vm_transpose(1, 2 * st + 1)
                if st == 8:
                    vs1 = vsum_calc(1)
                if st == 9:
                    vsump_calc(1, vs1)
                # counts0: ch = st//2, 4 pairs per slot (slots 0-7)
                if st < 8:
                    chc = st // 2
                    for j in range(4):
                        counts_pair(0, chc, 4 * (st % 2) + j)
                    if st % 2 == 1:
                        counts_evac(0, chc, cnt0)
                if st == 8:
                    w0, rgw0 = w_math(0, cnt0)
                # yv0: ch = st//4, 2 pairs per slot
                chy = st // 4
                yv_pair(0, chy, 2 * (st % 4))
                yv_pair(0, chy, 2 * (st % 4) + 1)
                if st % 4 == 3:
                    yv_evac(0, chy)
                raw_step(1, st)

            # tail: head-1 counts + Yv + w; then output + sliced RS
            nc.vector.tensor_copy(vm8[1][:], vm[1][:])
            cnt1 = work.tile([2, N], f16, tag="cnt1", bufs=1, name="cnt1")
            for ch in range(4):
                for sp in range(8):
                    counts_pair(1, ch, sp)
                counts_evac(1, ch, cnt1)
            w1, rgw1 = w_math(1, cnt1)
            for ch in range(4):
                for sp in range(8):
                    yv_pair(1, ch, sp)
                yv_evac(1, ch)
            wcol = [w0, w1]
            rgwT = [rgw0, rgw1]

            partialA = dram.tile([N, 512], f16)
            partialB = dram.tile([N, 512], f16)
            rsA = dram.tile([N // 4, 512], f16)
            rsB = dram.tile([N // 4, 512], f16)
            for cch in range(2):
                partial_d = partialA if cch == 0 else partialB
                rs_d = rsA if cch == 0 else rsB
                for half in range(2):
                    for nt in range(8 * half, 8 * half + 8):
                        ob = outp.tile([128, 512], f16, tag="ob", name="ob")
                        ps0 = ps_raw.tile([128, 512], f32, tag="r", name="ps0")
                        nc.tensor.matmul(ps0[:],
                                         rgwT[0][0:1, nt * 128:(nt + 1) * 128],
                                         vsump[0][0:1, cch * 512:(cch + 1) * 512],
                                         start=True, stop=False)
                        nc.tensor.matmul(ps0[:],
                                         S_sb[0][:, nt * 128:(nt + 1) * 128],
                                         wp_sb[:, 0 * C + cch * 512:
                                               0 * C + cch * 512 + 512],
                                         start=False, stop=True)
                        nc.scalar.activation(ob[:], ps0[:], AF.Copy, bias=0.0,
                                             scale=wcol[0][:, nt:nt + 1])
                        ps1 = ps_raw.tile([128, 512], f32, tag="r", name="ps1")
                        nc.tensor.matmul(ps1[:],
                                         rgwT[1][0:1, nt * 128:(nt + 1) * 128],
                                         vsump[1][0:1, cch * 512:(cch + 1) * 512],
                                         start=True, stop=False)
                        nc.tensor.matmul(ps1[:],
                                         S_sb[1][:, nt * 128:(nt + 1) * 128],
                                         wp_sb[:, 1 * C + cch * 512:
                                               1 * C + cch * 512 + 512],
                                         start=False, stop=True)
                        sc1 = outp.tile([128, 512], f16, tag="sc1", name="sc1")
                        nc.scalar.activation(sc1[:], ps1[:], AF.Copy, bias=0.0,
                                             scale=wcol[1][:, nt:nt + 1])
                        nc.vector.tensor_add(ob[:], ob[:], sc1[:])
                        nc.sync.dma_start(partial_d[nt * 128:(nt + 1) * 128, :],
                                          ob[:])
                    # RS on this half: rows [half*1024, half*1024+1024)
                    nc.gpsimd.collective_compute(
                        "ReduceScatter", OP.add,
                        replica_groups=[[0, 1, 2, 3], [4, 5, 6, 7]],
                        ins=[partial_d[half * 1024:(half + 1) * 1024, :].opt()],
                        outs=[rs_d[half * 256:(half + 1) * 256, :].opt()])
                    nc.gpsimd.dma_start(
                        d_out[half * 256:(half + 1) * 256,
                              cch * 512:(cch + 1) * 512],
                        rs_d[half * 256:(half + 1) * 256, :])

    nc.compile()
    return nc


def _host_inputs(query, mask, Wq, bq, Wk, bk, Wv, bv, Wp, bp):
    """Per-core input dicts (fp16 activations/weights, SCALE folded into Wq)."""
    f16 = np.float16
    f8 = ml_dtypes.float8_e4m3fn
    ins = []
    idt = np.eye(128, dtype=f16)
    idf = np.eye(128, dtype=np.float32)
    onesc = np.ones((128, 16), dtype=f16)
    for c in range(NCORES):
        b, g = c // 4, c % 4
        h0 = 2 * g
        sl = slice(h0 * HD, (h0 + 2) * HD)
        qt = np.ascontiguousarray(query[b].T).astype(f16)
        maskT = np.ascontiguousarray(
            mask[b].reshape(ST, 128).T.astype(np.float32))
        om8 = np.zeros((128, ST, 2), dtype=f8)
        om8[:, :, 0] = 1.0
        om8[:, :, 1] = maskT.astype(f8)
        mbcol = np.full((128, 1), float(mask[b].sum()), dtype=np.float32)
        bqkv = np.zeros((128, 6), dtype=np.float32)
        for i in range(2):
            bqkv[:, 0 + i] = SCALE * bq[(h0 + i) * HD:(h0 + i + 1) * HD]
            bqkv[:, 2 + i] = bk[(h0 + i) * HD:(h0 + i + 1) * HD]
            bqkv[:, 4 + i] = bv[(h0 + i) * HD:(h0 + i + 1) * HD]
        ins.append(dict(
            qt=qt,
            wq=np.ascontiguousarray(Wq[:, sl] * SCALE).astype(f16),
            wk=np.ascontiguousarray(Wk[:, sl]).astype(f16),
            wv=np.ascontiguousarray(Wv[:, sl]).astype(f16),
            wp=np.ascontiguousarray(Wp[sl, :]).astype(f16),
            maskT=maskT, om8=om8, onesc=onesc, mbcol=mbcol,
            idt=idt, idf=idf, bqkv=bqkv))
    return ins


def kernel(query, mask, Wq, bq, Wk, bk, Wv, bv, Wp, bp):
    from concourse.bass_utils import run_bass_kernel_spmd

    if "nc" not in _cache:
        _cache["nc"] = _build()
    nc = _cache["nc"]
    ins = _host_inputs(query, mask, Wq, bq, Wk, bk, Wv, bv, Wp, bp)
    res = run_bass_kernel_spmd(nc, ins, list(range(NCORES)))
    out = np.empty((B, N, C), dtype=np.float32)
    for b in range(B):
        for p in range(4):
            o = res.results[4 * b + p]["out"].astype(np.float32)
            # sliced RS: d_out rows 0-255 <- out[b] rows 256p..256p+256,
            #            rows 256-511 <- out[b] rows 1024+256p..
            out[b, 256 * p:256 * (p + 1)] = o[0:256]
            out[b, 1024 + 256 * p:1024 + 256 * (p + 1)] = o[256:512]
    out += np.asarray(bp, dtype=np.float32)[None, None, :]
    return out


# revision 12
# speedup vs baseline: 1.5283x; 1.0048x over previous
"""AssignAttention (topk_masking) Trainium2 kernel — 8 NeuronCores.

Sharding: data-parallel over B (2 groups of 4 cores), tensor-parallel over
heads H (2 heads per core). fp16 matmuls with f32 PSUM accumulation; Y and
the Y-consuming matmuls (counts, Yv) run in fp8e4 with DoubleRow perf mode.
Emission is software-pipelined: the QKV projection groups are interleaved
into head-0's raw/topk loop and head-0's counts/Yv matmuls into head-1's
raw/topk loop so Tensor/Scalar/Vector stay concurrently busy. The final
ReduceScatter is sliced 4x so it starts while output tiles are still being
computed.

Math: with Y[n,s] = [n in top4 of column s], c_n = sum_s Y, cm_n = sum_s
mask_s Y, e_n = exp(1/(c_n+1)), M = sum_s mask_s, Z_n = M + (e_n-1) cm_n:
out_head[n,:] = (Vsum + (e_n-1) * (Y.mask @ v)[n,:]) / Z_n
which equals the reference's hard-topk + sum-normalize + masked softmax.
"""
import sys, os

os.environ["JAX_ENABLE_COMPILATION_CACHE"] = "false"
sys.path.insert(0, "/opt/trn_rl_repo")
import numpy as np
import ml_dtypes

B, N, C, H, K = 2, 2048, 1024, 8, 4
HD = C // H
SCALE = HD ** -0.5
NCORES = 8
ST = 16          # s-tiles per head

_cache = {}


def _build():
    from concourse import bacc, tile, mybir

    f32, f16 = mybir.dt.float32, mybir.dt.float16
    f8 = mybir.dt.float8e4
    DR = mybir.MatmulPerfMode.DoubleRow
    AF = mybir.ActivationFunctionType
    OP = mybir.AluOpType

    nc = bacc.Bacc(None, target_bir_lowering=False)
    d_qt = nc.declare_dram_parameter("qt", [C, N], f16, isOutput=False)
    d_wq = nc.declare_dram_parameter("wq", [C, 2 * HD], f16, isOutput=False)
    d_wk = nc.declare_dram_parameter("wk", [C, 2 * HD], f16, isOutput=False)
    d_wv = nc.declare_dram_parameter("wv", [C, 2 * HD], f16, isOutput=False)
    d_wp = nc.declare_dram_parameter("wp", [2 * HD, C], f16, isOutput=False)
    d_maskT = nc.declare_dram_parameter("maskT", [128, ST], f32, isOutput=False)
    d_om = nc.declare_dram_parameter("om8", [128, ST, 2], f8, isOutput=False)
    d_ones = nc.declare_dram_parameter("onesc", [128, 16], f16, isOutput=False)
    d_mb = nc.declare_dram_parameter("mbcol", [128, 1], f32, isOutput=False)
    d_idt = nc.declare_dram_parameter("idt", [128, 128], f16, isOutput=False)
    d_idf = nc.declare_dram_parameter("idf", [128, 128], f32, isOutput=False)
    d_bqkv = nc.declare_dram_parameter("bqkv", [128, 6], f32, isOutput=False)
    d_out = nc.declare_dram_parameter("out", [N // 4, C], f16, isOutput=True)

    with tile.TileContext(nc) as tc:
        with (
            tc.tile_pool(name="cst", bufs=1) as cst,
            tc.tile_pool(name="proj", bufs=1) as proj,
            tc.tile_pool(name="ld", bufs=1) as ld,
            tc.tile_pool(name="yb", bufs=1) as yb,
            tc.tile_pool(name="work", bufs=2) as work,
            tc.tile_pool(name="outp", bufs=2) as outp,
            tc.tile_pool(name="ps_raw", bufs=5, space="PSUM") as ps_raw,
            tc.tile_pool(name="ps_yv", bufs=1, space="PSUM") as ps_yv,
            tc.tile_pool(name="ps_cnt", bufs=1, space="PSUM") as ps_cnt,
            tc.tile_pool(name="dram", bufs=1, space="DRAM") as dram,
        ):
            # ---- constants + inputs (DMAs spread over engine queues) ----
            maskT = cst.tile([128, ST], f32)
            om8 = cst.tile([128, ST, 2], f8)
            onesc = cst.tile([128, 16], f16)
            mbcol = cst.tile([128, 1], f32)
            idt = cst.tile([128, 128], f16)
            idf = cst.tile([128, 128], f32)
            bqkv = cst.tile([128, 6], f32)
            for t, d in [(maskT, d_maskT), (om8, d_om), (onesc, d_ones),
                         (mbcol, d_mb), (idt, d_idt), (idf, d_idf),
                         (bqkv, d_bqkv)]:
                nc.gpsimd.dma_start(t[:], d[:])
            wp_sb = cst.tile([128, 2 * C], f16)
            for h in range(2):
                nc.gpsimd.dma_start(wp_sb[:, h * C:(h + 1) * C],
                                    d_wp[h * 128:(h + 1) * 128, :])
            dumA = dram.tile([4, 512], f16)
            dumO = dram.tile([1, 512], f16)
            nc.gpsimd.collective_compute(
                "ReduceScatter", OP.add,
                replica_groups=[[0, 1, 2, 3], [4, 5, 6, 7]],
                ins=[dumA[:].opt()], outs=[dumO[:].opt()])
            qt_sb = ld.tile([128, 8 * N], f16)
            w_sb = {nm: ld.tile([128, 8 * 2 * HD], f16, tag=f"w{nm}",
                                name=f"wsb{nm}")
                    for nm in ("q", "k", "v")}
            qs = [nc.sync, nc.scalar, nc.gpsimd]
            for i in range(8):
                nc.sync.dma_start(w_sb["q"][:, i * 256:(i + 1) * 256],
                                  d_wq[i * 128:(i + 1) * 128, :])
                nc.scalar.dma_start(w_sb["k"][:, i * 256:(i + 1) * 256],
                                    d_wk[i * 128:(i + 1) * 128, :])
                qs[i % 3].dma_start(qt_sb[:, i * N:(i + 1) * N],
                                    d_qt[i * 128:(i + 1) * 128, :])
            for i in range(8):
                nc.gpsimd.dma_start(w_sb["v"][:, i * 256:(i + 1) * 256],
                                    d_wv[i * 128:(i + 1) * 128, :])

            qT = [proj.tile([128, N], f16, tag=f"q{h}", name=f"qT{h}")
                  for h in range(2)]
            kT = [proj.tile([128, N], f16, tag=f"k{h}", name=f"kT{h}")
                  for h in range(2)]
            vTb = [ld.tile([128, N], f16, tag=f"v{h}", name=f"vTb{h}")
                   for h in range(2)]
            vm = [proj.tile([128, N], f16, tag=f"vm{h}", name=f"vm{h}")
                  for h in range(2)]
            vm8 = [proj.tile([128, ST, 128], f8, tag=f"vm8{h}", name=f"vm8{h}")
                   for h in range(2)]
            vsump = [cst.tile([1, C], f16, tag=f"vsump{h}", name=f"vsump{h}")
                     for h in range(2)]
            S_sb = [proj.tile([128, N], f16, tag=f"s{h}", name=f"Ssb{h}")
                    for h in range(2)]
            ybig8 = [yb.tile([128, ST, N], f8, tag=f"y{h}", name=f"ybig{h}")
                     for h in range(2)]

            BI = {"q": 0, "k": 1, "v": 2}

            def proj_group(nm, h, ch):
                dst = {"q": qT, "k": kT, "v": vTb}[nm][h]
                ps = ps_raw.tile([128, 512], f32, tag="r", name="pg")
                for ci in range(8):
                    nc.tensor.matmul(
                        ps[:],
                        w_sb[nm][:, ci * 256 + h * 128:ci * 256 + (h + 1) * 128],
                        qt_sb[:, ci * N + ch * 512:ci * N + ch * 512 + 512],
                        start=(ci == 0), stop=(ci == 7))
                nc.scalar.activation(dst[:, ch * 512:(ch + 1) * 512], ps[:],
                                     AF.Identity,
                                     bias=bqkv[:, 2 * BI[nm] + h:2 * BI[nm] + h + 1],
                                     scale=1.0)

            def vm_transpose(h, st):
                ps = ps_cnt.tile([128, 128], f16, tag="tr", name="vt")
                nc.tensor.transpose(ps[:], vTb[h][:, st * 128:(st + 1) * 128],
                                    idt[:])
                nc.vector.tensor_scalar(vm[h][:, st * 128:(st + 1) * 128],
                                        ps[:], maskT[:, st:st + 1], None,
                                        OP.mult)

            def vsum_calc(h):
                pvs = ps_cnt.tile([128, 16], f32, tag="tr", name="pvs")
                for st in range(ST):
                    nc.tensor.matmul(pvs[:], vm[h][:, st * 128:(st + 1) * 128],
                                     onesc[:], start=(st == 0),
                                     stop=(st == ST - 1))
                vs = cst.tile([128, 1], f16, tag=f"vs{h}", name=f"vs{h}")
                nc.vector.tensor_copy(vs[:], pvs[:, 0:1])
                return vs

            def vsump_calc(h, vs):
                for ch in range(2):
                    pvp = ps_cnt.tile([1, 512], f32, tag="tr", name="pvp")
                    nc.tensor.matmul(pvp[:], vs[:],
                                     wp_sb[:, h * C + ch * 512:
                                           h * C + ch * 512 + 512],
                                     start=True, stop=True)
                    nc.vector.tensor_copy(vsump[h][0:1, ch * 512:(ch + 1) * 512],
                                          pvp[:])

            def raw_step(h, st):
                rawf = work.tile([128, N], f16, tag="raw", name="rawf")
                for ch in range(4):
                    ps = ps_raw.tile([128, 512], f32, tag="r", name="psr")
                    nc.tensor.matmul(ps[:], kT[h][:, st * 128:(st + 1) * 128],
                                     qT[h][:, ch * 512:(ch + 1) * 512],
                                     start=True, stop=True)
                    nc.scalar.activation(rawf[:, ch * 512:(ch + 1) * 512],
                                         ps[:], AF.Copy, bias=0.0, scale=1.0)
                top8 = work.tile([128, 8], f32, tag="top8", name="top8")
                nc.vector.max(top8[:], rawf[:])
                nc.vector.tensor_scalar(ybig8[h][:, st, :], rawf[:],
                                        top8[:, K - 1:K], None, OP.is_ge)

            cnt_ps = {}

            def counts_pair(h, ch, sp):
                # accumulate om8^T @ Y over s-tile pair sp (DoubleRow fp8)
                if sp == 0:
                    cnt_ps[h] = ps_cnt.tile([2, 512], f32, tag="c", name="pc")
                for t in (2 * sp, 2 * sp + 1):
                    nc.tensor.matmul(cnt_ps[h][:], om8[:, t, :],
                                     ybig8[h][:, t, ch * 512:ch * 512 + 512],
                                     start=(t == 0), stop=(t == 2 * ST - 1))

            def counts_evac(h, ch, cnt_sb):
                nc.vector.tensor_copy(cnt_sb[:, ch * 512:(ch + 1) * 512],
                                      cnt_ps[h][:])

            yv_ps = {}

            def yv_pair(h, ch, sp):
                if sp == 0:
                    yv_ps[h] = ps_yv.tile([128, 512], f32, tag="yv", name="py")
                nc.tensor.matmul(yv_ps[h][:], vm8[h][:, 2 * sp:2 * sp + 2, :],
                                 ybig8[h][:, 2 * sp:2 * sp + 2,
                                          ch * 512:ch * 512 + 512],
                                 start=(sp == 0), stop=(sp == 7), perf_mode=DR)

            def yv_evac(h, ch):
                nc.scalar.activation(S_sb[h][:, ch * 512:(ch + 1) * 512],
                                     yv_ps[h][:], AF.Copy, bias=0.0, scale=1.0)

            def w_math(h, cnt_sb):
                ptr = ps_cnt.tile([128, 32], f16, tag="tr", name="ptr")
                for t2 in range(ST):
                    nc.tensor.transpose(ptr[:, 2 * t2:2 * t2 + 2],
                                        cnt_sb[:, t2 * 128:(t2 + 1) * 128],
                                        idt[:2, :2])
                cntT = work.tile([128, 32], f32, tag="cntT", name="cntT")
                nc.vector.tensor_copy(cntT[:], ptr[:])
                rec = work.tile([128, 16], f32, tag="rec", name="rec")
                nc.vector.tensor_scalar(rec[:], cntT[:, 0:32:2], 1.0, None,
                                        OP.add)
                nc.vector.reciprocal(rec[:], rec[:])
                e = work.tile([128, 16], f32, tag="e", name="e")
                nc.scalar.activation(e[:], rec[:], AF.Exp)
                em1 = work.tile([128, 16], f32, tag="em1", name="em1")
                nc.vector.tensor_scalar(em1[:], e[:], -1.0, None, OP.add)
                Z = work.tile([128, 16], f32, tag="Z", name="Zt")
                nc.vector.tensor_mul(Z[:], em1[:], cntT[:, 1:32:2])
                nc.vector.tensor_scalar(Z[:], Z[:], mbcol[:, 0:1], None, OP.add)
                r_ = work.tile([128, 16], f32, tag="r_", name="r_")
                nc.vector.reciprocal(r_[:], Z[:])
                w_ = work.tile([128, 16], f32, tag=f"w{h}_", name=f"w{h}_")
                nc.vector.tensor_mul(w_[:], r_[:], em1[:])
                rem = work.tile([128, 16], f32, tag="rem", name="rem")
                nc.vector.reciprocal(rem[:], em1[:])
                prt = ps_cnt.tile([16, 128], f32, tag="tr", name="prt")
                nc.tensor.transpose(prt[:], rem[:], idf[:])
                rgt16 = work.tile([16, 128], f16, tag="rgT16", bufs=1,
                                  name="rgt16")
                nc.vector.tensor_copy(rgt16[:], prt[:])
                rgt = work.tile([1, 16 * 128], f16, tag=f"rgTf{h}",
                                name=f"rgTf{h}")
                nc.sync.dma_start(rgt[:], rgt16[:])
                return w_, rgt

            # ================= schedule =================
            # pre: q0 + k0 projection groups
            for ch in range(4):
                proj_group("q", 0, ch)
            for ch in range(4):
                proj_group("k", 0, ch)

            # A0: head-0 raw/topk, interleaved with remaining projections
            PG = ([("v", 0, ch) for ch in range(4)] +
                  [("q", 1, ch) for ch in range(4)] +
                  [("k", 1, ch) for ch in range(4)] +
                  [("v", 1, ch) for ch in range(4)])
            vs0 = None
            for st in range(ST):
                proj_group(*PG[st])
                if 4 <= st < 12:
                    vm_transpose(0, 2 * (st - 4))
                    vm_transpose(0, 2 * (st - 4) + 1)
                if st == 12:
                    vs0 = vsum_calc(0)
                if st == 13:
                    vsump_calc(0, vs0)
                if st == 14:
                    nc.vector.tensor_copy(vm8[0][:], vm[0][:])
                raw_step(0, st)

            # A1: head-1 raw/topk, interleaved with head-0 counts + Yv
            cnt0 = work.tile([2, N], f16, tag="cnt", bufs=1, name="cnt0")
            w0 = rgw0 = None
            vs1 = None
            for st in range(ST):
                if st < 8:
                    vm_transpose(1, 2 * st)
                    vm_transpose(1, 2 * st + 1)
                if st == 8:
                    vs1 = vsum_calc(1)
                if st == 9:
                    vsump_calc(1, vs1)
                # counts0: ch = st//2, 4 pairs per slot (slots 0-7)
                if st < 8:
                    chc = st // 2
                    for j in range(4):
                        counts_pair(0, chc, 4 * (st % 2) + j)
                    if st % 2 == 1:
                        counts_evac(0, chc, cnt0)
                if st == 8:
                    w0, rgw0 = w_math(0, cnt0)
                # yv0: ch = st//4, 2 pairs per slot
                chy = st // 4
                yv_pair(0, chy, 2 * (st % 4))
                yv_pair(0, chy, 2 * (st % 4) + 1)
                if st % 4 == 3:
                    yv_evac(0, chy)
                raw_step(1, st)

            # tail: head-1 counts + Yv + w; then output + sliced RS
            nc.vector.tensor_copy(vm8[1][:], vm[1][:])
            cnt1 = work.tile([2, N], f16, tag="cnt1", bufs=1, name="cnt1")
            for ch in range(4):
                for sp in range(8):
                    counts_pair(1, ch, sp)
                counts_evac(1, ch, cnt1)
            w1, rgw1 = w_math(1, cnt1)
            for ch in range(4):
                for sp in range(8):
                    yv_pair(1, ch, sp)
                yv_evac(1, ch)
            wcol = [w0, w1]
            rgwT = [rgw0, rgw1]

            partialA = dram.tile([N, 512], f16)
            partialB = dram.tile([N, 512], f16)
            rsA = dram.tile([N // 4, 512], f16)
            rsB = dram.tile([N // 4, 512], f16)
            for cch in range(2):
                partial_d = partialA if cch == 0 else partialB
                rs_d = rsA if cch == 0 else rsB
                for half in range(4):
                    for nt in range(4 * half, 4 * half + 4):
                        ob = outp.tile([128, 512], f16, tag="ob", name="ob")
                        ps0 = ps_raw.tile([128, 512], f32, tag="r", name="ps0")
                        nc.tensor.matmul(ps0[:],
                                         rgwT[0][0:1, nt * 128:(nt + 1) * 128],
                                         vsump[0][0:1, cch * 512:(cch + 1) * 512],
                                         start=True, stop=False)
                        nc.tensor.matmul(ps0[:],
                                         S_sb[0][:, nt * 128:(nt + 1) * 128],
                                         wp_sb[:, 0 * C + cch * 512:
                                               0 * C + cch * 512 + 512],
                                         start=False, stop=True)
                        nc.scalar.activation(ob[:], ps0[:], AF.Copy, bias=0.0,
                                             scale=wcol[0][:, nt:nt + 1])
                        ps1 = ps_raw.tile([128, 512], f32, tag="r", name="ps1")
                        nc.tensor.matmul(ps1[:],
                                         rgwT[1][0:1, nt * 128:(nt + 1) * 128],
                                         vsump[1][0:1, cch * 512:(cch + 1) * 512],
                                         start=True, stop=False)
                        nc.tensor.matmul(ps1[:],
                                         S_sb[1][:, nt * 128:(nt + 1) * 128],
                                         wp_sb[:, 1 * C + cch * 512:
                                               1 * C + cch * 512 + 512],
                                         start=False, stop=True)
                        sc1 = outp.tile([128, 512], f16, tag="sc1", name="sc1")
                        nc.scalar.activation(sc1[:], ps1[:], AF.Copy, bias=0.0,
                                             scale=wcol[1][:, nt:nt + 1])
                        nc.vector.tensor_add(ob[:], ob[:], sc1[:])
                        nc.sync.dma_start(partial_d[nt * 128:(nt + 1) * 128, :],
                                          ob[:])
                    # RS on this slice: rows [half*512, half*512+512)
                    nc.gpsimd.collective_compute(
                        "ReduceScatter", OP.add,
                        replica_groups=[[0, 1, 2, 3], [4, 5, 6, 7]],
                        ins=[partial_d[half * 512:(half + 1) * 512, :].opt()],
                        outs=[rs_d[half * 128:(half + 1) * 128, :].opt()])
                    nc.gpsimd.dma_start(
                        d_out[half * 128:(half + 1) * 128,
                              cch * 512:(cch + 1) * 512],
                        rs_d[half * 128:(half + 1) * 128, :])

    nc.compile()
    return nc


def _host_inputs(query, mask, Wq, bq, Wk, bk, Wv, bv, Wp, bp):
    """Per-core input dicts (fp16 activations/weights, SCALE folded into Wq)."""
    f16 = np.float16
    f8 = ml_dtypes.float8_e4m3fn
    ins = []
    idt = np.eye(128, dtype=f16)
    idf = np.eye(128, dtype=np.float32)
    onesc = np.ones((128, 16), dtype=f16)
    for c in range(NCORES):
        b, g = c // 4, c % 4
        h0 = 2 * g
        sl = slice(h0 * HD, (h0 + 2) * HD)
        qt = np.ascontiguousarray(query[b].T).astype(f16)
        maskT = np.ascontiguousarray(
            mask[b].reshape(ST, 128).T.astype(np.float32))
        om8 = np.zeros((128, ST, 2), dtype=f8)
        om8[:, :, 0] = 1.0
        om8[:, :, 1] = maskT.astype(f8)
        mbcol = np.full((128, 1), float(mask[b].sum()), dtype=np.float32)
        bqkv = np.zeros((128, 6), dtype=np.float32)
        for i in range(2):
            bqkv[:, 0 + i] = SCALE * bq[(h0 + i) * HD:(h0 + i + 1) * HD]
            bqkv[:, 2 + i] = bk[(h0 + i) * HD:(h0 + i + 1) * HD]
            bqkv[:, 4 + i] = bv[(h0 + i) * HD:(h0 + i + 1) * HD]
        ins.append(dict(
            qt=qt,
            wq=np.ascontiguousarray(Wq[:, sl] * SCALE).astype(f16),
            wk=np.ascontiguousarray(Wk[:, sl]).astype(f16),
            wv=np.ascontiguousarray(Wv[:, sl]).astype(f16),
            wp=np.ascontiguousarray(Wp[sl, :]).astype(f16),
            maskT=maskT, om8=om8, onesc=onesc, mbcol=mbcol,
            idt=idt, idf=idf, bqkv=bqkv))
    return ins


def kernel(query, mask, Wq, bq, Wk, bk, Wv, bv, Wp, bp):
    from concourse.bass_utils import run_bass_kernel_spmd

    if "nc" not in _cache:
        _cache["nc"] = _build()
    nc = _cache["nc"]
    ins = _host_inputs(query, mask, Wq, bq, Wk, bk, Wv, bv, Wp, bp)
    res = run_bass_kernel_spmd(nc, ins, list(range(NCORES)))
    out = np.empty((B, N, C), dtype=np.float32)
    for b in range(B):
        for p in range(4):
            o = res.results[4 * b + p]["out"].astype(np.float32)
            # 8-way sliced RS: d_out rows 128j..128j+128 <- out[b] rows
            # 512j + 128p .. 512j + 128(p+1)
            for j in range(4):
                out[b, 512 * j + 128 * p:512 * j + 128 * (p + 1)] = \
                    o[128 * j:128 * (j + 1)]
    out += np.asarray(bp, dtype=np.float32)[None, None, :]
    return out
